# Initial kernel scaffold
#
import sys
sys.path.insert(0, '/opt/trn_rl_repo')
import numpy as np
import concourse.bass as bass
import concourse.bacc as bacc
import concourse.mybir as mybir
import concourse.tile as tile
from concourse.bass import IndirectOffsetOnAxis
from concourse.bass_utils import run_bass_kernel_spmd

P = 128
T = 1024
S = 1024
D = 512
H = 8
DK = 64
DFF = 2048
VOC = 32000
NT = T // P   # 8 token tiles
ND = D // P   # 4 d-model chunks
NJ = DFF // P  # 16 dff tiles
L_FULL = 6
EPS = 1e-5
NEG = -1e9

F32 = mybir.dt.float32
F32R = mybir.dt.float32r
I32 = mybir.dt.int32
AF = mybir.ActivationFunctionType
OP = mybir.AluOpType


def _pe_table():
    pos = np.arange(T)[:, None].astype(np.float64)
    div = np.exp(np.arange(0, D, 2).astype(np.float64) * (-np.log(10000.0) / D))
    pe = np.zeros((T, D))
    pe[:, 0::2] = np.sin(pos * div)
    pe[:, 1::2] = np.cos(pos * div)
    return pe.astype(np.float32)


def _causal_master():
    kk = np.arange(P)[:, None]
    u = np.arange(896)[None, :]
    return np.where(kk > u - 384, NEG, 0.0).astype(np.float32)


def build(n_layers=L_FULL):
    nc = bacc.Bacc("TRN2", target_bir_lowering=False, debug=False, num_devices=8)

    embd = nc.dram_tensor("emb", [VOC, D], F32, kind="ExternalInput")
    decd = nc.dram_tensor("dec_idx", [P, NT], I32, kind="ExternalInput")
    encd = nc.dram_tensor("enc_idx", [P, NT], I32, kind="ExternalInput")
    ped = nc.dram_tensor("pe", [T, D], F32, kind="ExternalInput")
    causd = nc.dram_tensor("causal", [P, 896], F32, kind="ExternalInput")
    identd = nc.dram_tensor("ident", [P, P], F32, kind="ExternalInput")
    encxd = nc.dram_tensor("encx", [S, D], F32, kind="ExternalInput")
    WQS = nc.dram_tensor("wq_s", [n_layers, D, D], F32R, kind="ExternalInput")
    WKS = nc.dram_tensor("wk_s", [n_layers, D, D], F32R, kind="ExternalInput")
    WVS = nc.dram_tensor("wv_s", [n_layers, D, D], F32R, kind="ExternalInput")
    WOS = nc.dram_tensor("wo_s", [n_layers, D, D], F32R, kind="ExternalInput")
    WQC = nc.dram_tensor("wq_c", [n_layers, D, D], F32R, kind="ExternalInput")
    WKC = nc.dram_tensor("wk_c", [n_layers, D, D], F32R, kind="ExternalInput")
    WVC = nc.dram_tensor("wv_c", [n_layers, D, D], F32R, kind="ExternalInput")
    WOC = nc.dram_tensor("wo_c", [n_layers, D, D], F32R, kind="ExternalInput")
    W1D = nc.dram_tensor("w1", [n_layers, D, DFF], F32R, kind="ExternalInput")
    W2D = nc.dram_tensor("w2", [n_layers, DFF, D], F32R, kind="ExternalInput")
    outd = nc.dram_tensor("out", [T, D], F32, kind="ExternalOutput")

    with nc.allow_low_precision(reason="f32r rounding intended"), \
         tile.TileContext(nc) as tc:
        with tc.tile_pool(name="pers", bufs=1) as pers, \
             tc.tile_pool(name="dbl", bufs=2) as dbl, \
             tc.tile_pool(name="p3", bufs=3) as p3, \
             tc.tile_pool(name="p5", bufs=5) as p5, \
             tc.tile_pool(name="p4", bufs=4) as p4, \
             tc.tile_pool(name="psS", bufs=3, space="PSUM") as psS, \
             tc.tile_pool(name="psB", bufs=4, space="PSUM") as psB, \
             tc.tile_pool(name="psR", bufs=1, space="PSUM") as psR:

            # ---------------- persistent tiles ----------------
            x_res = pers.tile([P, NT, D], F32)       # [tok_in_tile, t_tile, D]
            xT = pers.tile([P, ND, T], F32R)         # [d_in_chunk, d_chunk, tok]
            encT = pers.tile([P, ND, S], F32R)
            QT = pers.tile([P, ND, T], F32R)
            KT = pers.tile([P, ND, T], F32R)
            vext = pers.tile([P, NT, H, DK + 1], F32R)  # [k_in_tile, k_tile, head, dv+1]
            causal_sb = pers.tile([P, 896], F32)
            ident_sb = pers.tile([P, P], F32)
            ones_f = pers.tile([P, 64], F32)
            ones64 = pers.tile([1, 64], F32R)
            eps_sb = pers.tile([P, 1], F32)
            dec_sb = pers.tile([P, NT], I32)
            enc_sb = pers.tile([P, NT], I32)
            bias_dec = pers.tile([P, NT], F32)
            bias_enc = pers.tile([P, NT], F32)
            mscale = pers.tile([P, NT], F32)

            nc.sync.dma_start(out=causal_sb, in_=causd[:, :])
            nc.sync.dma_start(out=ident_sb, in_=identd[:, :])
            nc.sync.dma_start(out=dec_sb, in_=decd[:, :])
            nc.sync.dma_start(out=enc_sb, in_=encd[:, :])
            nc.vector.memset(ones_f, 1.0)
            nc.vector.memset(eps_sb, EPS)
            nc.vector.tensor_copy(out=ones64, in_=ones_f[0:1, :])

            # pad masks from token ids
            for tok_sb, bias_sb, want_mscale in (
                (dec_sb, bias_dec, True),
                (enc_sb, bias_enc, False),
            ):
                tokf = p4.tile([P, NT], F32, tag="tokf")
                nc.vector.tensor_copy(out=tokf, in_=tok_sb)
                is0 = p4.tile([P, NT], F32, tag="is0")
                nc.vector.tensor_scalar(out=is0, in0=tokf, scalar1=0.0,
                                        scalar2=None, op0=OP.is_equal)
                nc.vector.tensor_scalar(out=bias_sb, in0=is0, scalar1=NEG,
                                        scalar2=None, op0=OP.mult)
                if want_mscale:
                    nc.scalar.activation(out=mscale, in_=is0, func=AF.Copy,
                                         bias=1.0, scale=-1.0)

            # ones column of vext (written once; V eviction never touches it)
            for i in range(NT):
                nc.vector.tensor_copy(
                    out=vext[:, i, :, DK:DK + 1],
                    in_=ones_f[:, 0:H].rearrange("p (h o) -> p h o", o=1))

            # ---------------- embedding + pe ----------------
            for tt in range(NT):
                g = p3.tile([P, D], F32, tag="tmp")
                nc.gpsimd.indirect_dma_start(
                    out=g, out_offset=None, in_=embd[:, :],
                    in_offset=IndirectOffsetOnAxis(ap=dec_sb[:, tt:tt + 1], axis=0))
                pe_t = p3.tile([P, D], F32, tag="tmp")
                nc.sync.dma_start(out=pe_t, in_=ped[tt * P:(tt + 1) * P, :])
                g2 = p3.tile([P, D], F32, tag="tmp")
                nc.vector.tensor_scalar(out=g2, in0=g, scalar1=mscale[:, tt:tt + 1],
                                        scalar2=None, op0=OP.mult)
                nc.vector.tensor_add(out=x_res[:, tt, :], in0=g2, in1=pe_t)

            def transpose_to(dst, src_tile, tt_range=range(NT)):
                # dst: [P, ND, T] f32r; src_tile(tt) -> [P, D] f32 AP
                for tt in tt_range:
                    for d in range(ND):
                        ps_t = psB.tile([P, P], F32, tag="b")
                        nc.tensor.transpose(out=ps_t,
                                            in_=src_tile(tt)[:, d * P:(d + 1) * P],
                                            identity=ident_sb)
                        nc.vector.tensor_copy(
                            out=dst[:, d, tt * P:(tt + 1) * P], in_=ps_t)

            transpose_to(xT, lambda tt: x_res[:, tt, :])

            for tt in range(NT):
                e_t = p3.tile([P, D], F32, tag="tmp")
                nc.sync.dma_start(out=e_t, in_=encxd[tt * P:(tt + 1) * P, :])
                for d in range(ND):
                    ps_t = psB.tile([P, P], F32, tag="b")
                    nc.tensor.transpose(out=ps_t, in_=e_t[:, d * P:(d + 1) * P],
                                        identity=ident_sb)
                    nc.vector.tensor_copy(out=encT[:, d, tt * P:(tt + 1) * P],
                                          in_=ps_t)

            # ---------------- helpers ----------------
            def ln_into_xres(ps_in, tt):
                pre = p3.tile([P, D], F32, tag="tmp")
                nc.vector.tensor_add(out=pre, in0=ps_in, in1=x_res[:, tt, :])
                st = p4.tile([P, nc.vector.BN_STATS_DIM], F32, tag="st")
                nc.vector.bn_stats(out=st, in_=pre)
                mv = p4.tile([P, nc.vector.BN_AGGR_DIM], F32, tag="mv")
                nc.vector.bn_aggr(out=mv, in_=st)
                std = p4.tile([P, 1], F32, tag="sd")
                nc.scalar.activation(out=std, in_=mv[:, 1:2], func=AF.Sqrt,
                                     bias=eps_sb, scale=1.0)
                rstd = p4.tile([P, 1], F32, tag="rs")
                nc.vector.reciprocal(out=rstd, in_=std)
                nc.vector.tensor_scalar(out=x_res[:, tt, :], in0=pre,
                                        scalar1=mv[:, 0:1], scalar2=rstd,
                                        op0=OP.subtract, op1=OP.mult)

            def load_wattn(wd, l):
                w = dbl.tile([P, ND, D], F32R, tag="wattn")
                nc.sync.dma_start(
                    out=w, in_=wd[l].rearrange("(kc kp) n -> kp kc n", kp=P))
                return w

            def attn(l, is_self):
                wq = load_wattn(WQS if is_self else WQC, l)
                wk = load_wattn(WKS if is_self else WKC, l)
                wv = load_wattn(WVS if is_self else WVC, l)
                wo = load_wattn(WOS if is_self else WOC, l)
                kv = xT if is_self else encT
                bias_sb = bias_dec if is_self else bias_enc

                # QT / KT projections: psum [dq_tile, t_chunk]
                for dst, w, src in ((QT, wq, xT), (KT, wk, kv)):
                    for dq in range(ND):
                        for c in range(2):
                            ps = psS.tile([P, 512], F32, tag="s")
                            for kc in range(ND):
                                nc.tensor.matmul(
                                    ps, w[:, kc, dq * P:(dq + 1) * P],
                                    src[:, kc, c * 512:(c + 1) * 512],
                                    start=(kc == 0), stop=(kc == ND - 1))
                            nc.vector.tensor_copy(
                                out=dst[:, dq, c * 512:(c + 1) * 512], in_=ps)

                # V projection: psum [k_tile(tokens), dv 512] -> vext
                for i in range(NT):
                    ps = psS.tile([P, 512], F32, tag="s")
                    for kc in range(ND):
                        nc.tensor.matmul(ps, kv[:, kc, i * P:(i + 1) * P],
                                         wv[:, kc, :],
                                         start=(kc == 0), stop=(kc == ND - 1))
                    nc.vector.tensor_copy(
                        out=vext[:, i, :, 0:DK],
                        in_=ps.rearrange("p (h v) -> p h v", h=H))

                # scores -> exp -> AV, per (q_chunk, head pair)
                for c in range(2):
                    ctx_pairs = [p5.tile([P, 512], F32R, tag="ctx")
                                 for _ in range(ND)]
                    for d in range(ND):
                        for hh in range(2):
                            h = 2 * d + hh
                            hsl = slice(hh * 64, (hh + 1) * 64)
                            klist = list(range(4 * (c + 1))) if is_self \
                                else list(range(NT))
                            ps_ctx = psB.tile([DK + 1, 512], F32, tag="b")
                            for ki, i in enumerate(klist):
                                ps_s = psS.tile([P, 512], F32, tag="s")
                                nc.tensor.matmul(
                                    ps_s, KT[hsl, d, i * P:(i + 1) * P],
                                    QT[hsl, d, c * 512:(c + 1) * 512],
                                    start=True, stop=True)
                                if is_self and i >= 4 * c:
                                    r = i - 4 * c
                                    nc.vector.tensor_add(
                                        out=ps_s, in0=ps_s,
                                        in1=causal_sb[:, 384 - 128 * r:
                                                      384 - 128 * r + 512])
                                e = p3.tile([P, 512], F32R, tag="exp")
                                nc.scalar.activation(
                                    out=e, in_=ps_s, func=AF.Exp,
                                    bias=bias_sb[:, i:i + 1], scale=0.125)
                                nc.tensor.matmul(
                                    ps_ctx, vext[:, i, h, :], e,
                                    start=(ki == 0), stop=(ki == len(klist) - 1))
                            recip = p3.tile([1, 512], F32R, tag="recip")
                            nc.vector.reciprocal(out=recip,
                                                 in_=ps_ctx[DK:DK + 1, :])
                            ps_r = psR.tile([64, 512], F32, tag="r")
                            nc.tensor.matmul(ps_r, ones64, recip,
                                             start=True, stop=True)
                            ctxe = p3.tile([64, 512], F32, tag="ctxe")
                            nc.vector.tensor_copy(out=ctxe, in_=ps_ctx[0:DK, :])
                            nc.vector.tensor_mul(out=ctx_pairs[d][hsl, :],
                                                 in0=ctxe, in1=ps_r)
                    # output projection + residual + LN for this chunk
                    for ts_ in range(4):
                        tt = 4 * c + ts_
                        ps_o = psB.tile([P, 512], F32, tag="b")
                        for d in range(ND):
                            nc.tensor.matmul(
                                ps_o, ctx_pairs[d][:, ts_ * P:(ts_ + 1) * P],
                                wo[:, d, :], start=(d == 0), stop=(d == ND - 1))
                        ln_into_xres(ps_o, tt)
                transpose_to(xT, lambda tt: x_res[:, tt, :])

            def ffn(l, last):
                for c in range(2):
                    ps_fs = [psB.tile([P, 512], F32, tag="b") for _ in range(4)]
                    for j in range(NJ):
                        if j % 2 == 0:
                            jc = j // 2
                            w1t = dbl.tile([P, ND, 256], F32R, tag="w1c")
                            nc.sync.dma_start(
                                out=w1t,
                                in_=W1D[l][:, jc * 256:(jc + 1) * 256]
                                .rearrange("(kc kp) n -> kp kc n", kp=P))
                            w2t = dbl.tile([P, 2, D], F32R, tag="w2c")
                            nc.sync.dma_start(
                                out=w2t,
                                in_=W2D[l][jc * 256:(jc + 1) * 256, :]
                                .rearrange("(jj kp) n -> kp jj n", kp=P))
                        ps_h = psS.tile([P, 512], F32, tag="s")
                        for kc in range(ND):
                            nc.tensor.matmul(
                                ps_h, w1t[:, kc, (j % 2) * P:(j % 2 + 1) * P],
                                xT[:, kc, c * 512:(c + 1) * 512],
                                start=(kc == 0), stop=(kc == ND - 1))
                        hT = p3.tile([P, 512], F32R, tag="hT")
                        nc.scalar.activation(out=hT, in_=ps_h, func=AF.Relu)
                        for ts_ in range(4):
                            nc.tensor.matmul(
                                ps_fs[ts_], hT[:, ts_ * P:(ts_ + 1) * P],
                                w2t[:, j % 2, :],
                                start=(j == 0), stop=(j == NJ - 1))
                    for ts_ in range(4):
                        ln_into_xres(ps_fs[ts_], 4 * c + ts_)
                    if not last:
                        transpose_to(xT, lambda tt: x_res[:, tt, :],
                                     tt_range=range(4 * c, 4 * c + 4))

            # ---------------- layers ----------------
            for l in range(n_layers):
                attn(l, True)
                attn(l, False)
                ffn(l, last=(l == n_layers - 1))

            for tt in range(NT):
                nc.sync.dma_start(out=outd[tt * P:(tt + 1) * P, :],
                                  in_=x_res[:, tt, :])

    nc.compile()
    return nc


_CACHE = {}


def get_nc(n_layers=L_FULL):
    if n_layers not in _CACHE:
        _CACHE[n_layers] = build(n_layers)
    return _CACHE[n_layers]


def make_in_maps(dec_inputs, enc_inputs, enc_outputs, emb,
                 Wq_self, Wk_self, Wv_self, Wo_self,
                 Wq_cross, Wk_cross, Wv_cross, Wo_cross, W1, W2,
                 n_layers=L_FULL):
    f = np.ascontiguousarray
    emb = f(np.asarray(emb, dtype=np.float32))
    dec = np.asarray(dec_inputs).astype(np.int32)
    enc = np.asarray(enc_inputs).astype(np.int32)
    encx = f(np.asarray(enc_outputs, dtype=np.float32))
    pe = _pe_table()
    caus = _causal_master()
    ident = np.eye(P, dtype=np.float32)
    ws = {}
    for name, w in (("wq_s", Wq_self), ("wk_s", Wk_self), ("wv_s", Wv_self),
                    ("wo_s", Wo_self), ("wq_c", Wq_cross), ("wk_c", Wk_cross),
                    ("wv_c", Wv_cross), ("wo_c", Wo_cross), ("w1", W1),
                    ("w2", W2)):
        ws[name] = f(np.asarray(w, dtype=np.float32)[:n_layers])
    B = dec.shape[0]
    in_maps = []
    for b in range(B):
        m = dict(emb=emb, pe=pe, causal=caus, ident=ident,
                 dec_idx=f(dec[b].reshape(NT, P).T),
                 enc_idx=f(enc[b].reshape(NT, P).T),
                 encx=encx[b], **ws)
        in_maps.append(m)
    return in_maps


def kernel(**inputs):
    n_layers = inputs.pop("_n_layers", L_FULL)
    nc = get_nc(n_layers)
    in_maps = make_in_maps(**inputs, n_layers=n_layers)
    res = run_bass_kernel_spmd(nc, in_maps, core_ids=list(range(len(in_maps))))
    out = np.stack([r["out"] for r in res.results], axis=0)
    return out


# revision 4
# speedup vs baseline: 1.1252x; 1.1252x over previous
import sys
sys.path.insert(0, '/opt/trn_rl_repo')
import numpy as np
import concourse.bass as bass
import concourse.bacc as bacc
import concourse.mybir as mybir
import concourse.tile as tile
from concourse.bass import IndirectOffsetOnAxis
from concourse.bass_utils import run_bass_kernel_spmd

P = 128
T = 1024
S = 1024
D = 512
H = 8
DK = 64
DFF = 2048
VOC = 32000
NT = T // P   # 8 token tiles
ND = D // P   # 4 d-model chunks
NJ = DFF // P  # 16 dff tiles
L_FULL = 6
EPS = 1e-5
NEG = -1e9

F32 = mybir.dt.float32
F32R = mybir.dt.float32r
I32 = mybir.dt.int32
AF = mybir.ActivationFunctionType
OP = mybir.AluOpType


def _pe_table():
    pos = np.arange(T)[:, None].astype(np.float64)
    div = np.exp(np.arange(0, D, 2).astype(np.float64) * (-np.log(10000.0) / D))
    pe = np.zeros((T, D))
    pe[:, 0::2] = np.sin(pos * div)
    pe[:, 1::2] = np.cos(pos * div)
    return pe.astype(np.float32)


def _causal_master():
    kk = np.arange(P)[:, None]
    u = np.arange(896)[None, :]
    return np.where(kk > u - 384, NEG, 0.0).astype(np.float32)


def build(n_layers=L_FULL, repeat=1):
    nc = bacc.Bacc("TRN2", target_bir_lowering=False, debug=False, num_devices=8)

    embd = nc.dram_tensor("emb", [VOC, D], F32, kind="ExternalInput")
    decd = nc.dram_tensor("dec_idx", [P, NT], I32, kind="ExternalInput")
    encd = nc.dram_tensor("enc_idx", [P, NT], I32, kind="ExternalInput")
    ped = nc.dram_tensor("pe", [T, D], F32, kind="ExternalInput")
    causd = nc.dram_tensor("causal", [P, 896], F32, kind="ExternalInput")
    identd = nc.dram_tensor("ident", [P, P], F32, kind="ExternalInput")
    encxd = nc.dram_tensor("encx", [S, D], F32, kind="ExternalInput")
    WQS = nc.dram_tensor("wq_s", [n_layers, D, D], F32R, kind="ExternalInput")
    WKS = nc.dram_tensor("wk_s", [n_layers, D, D], F32R, kind="ExternalInput")
    WVS = nc.dram_tensor("wv_s", [n_layers, D, D], F32R, kind="ExternalInput")
    WOS = nc.dram_tensor("wo_s", [n_layers, D, D], F32R, kind="ExternalInput")
    WQC = nc.dram_tensor("wq_c", [n_layers, D, D], F32R, kind="ExternalInput")
    WKC = nc.dram_tensor("wk_c", [n_layers, D, D], F32R, kind="ExternalInput")
    WVC = nc.dram_tensor("wv_c", [n_layers, D, D], F32R, kind="ExternalInput")
    WOC = nc.dram_tensor("wo_c", [n_layers, D, D], F32R, kind="ExternalInput")
    W1D = nc.dram_tensor("w1", [n_layers, D, DFF], F32R, kind="ExternalInput")
    W2D = nc.dram_tensor("w2", [n_layers, DFF, D], F32R, kind="ExternalInput")
    outd = nc.dram_tensor("out", [T, D], F32, kind="ExternalOutput")

    with nc.allow_low_precision(reason="f32r rounding intended"), \
         tile.TileContext(nc) as tc:
        with tc.tile_pool(name="pers", bufs=1) as pers, \
             tc.tile_pool(name="dbl", bufs=2) as dbl, \
             tc.tile_pool(name="p3", bufs=3) as p3, \
             tc.tile_pool(name="p5", bufs=5) as p5, \
             tc.tile_pool(name="p4", bufs=4) as p4, \
             tc.tile_pool(name="psS", bufs=3, space="PSUM") as psS, \
             tc.tile_pool(name="psB", bufs=4, space="PSUM") as psB, \
             tc.tile_pool(name="psR", bufs=1, space="PSUM") as psR:

            # ---------------- persistent tiles ----------------
            x_res = pers.tile([P, NT, D], F32)       # [tok_in_tile, t_tile, D]
            xT = pers.tile([P, ND, T], F32R)         # [d_in_chunk, d_chunk, tok]
            encT = pers.tile([P, ND, S], F32R)
            QT = pers.tile([P, ND, T], F32R)
            KT = pers.tile([P, ND, T], F32R)
            vext = pers.tile([P, NT, H, DK + 1], F32R)  # [k_in_tile, k_tile, head, dv+1]
            causal_sb = pers.tile([P, 896], F32)
            ident_sb = pers.tile([P, P], F32)
            ones_f = pers.tile([P, 64], F32)
            ones64 = pers.tile([1, 64], F32R)
            eps_sb = pers.tile([P, 1], F32)
            dec_sb = pers.tile([P, NT], I32)
            enc_sb = pers.tile([P, NT], I32)
            bias_dec = pers.tile([P, NT], F32)
            bias_enc = pers.tile([P, NT], F32)
            mscale = pers.tile([P, NT], F32)

            nc.sync.dma_start(out=causal_sb, in_=causd[:, :])
            nc.sync.dma_start(out=ident_sb, in_=identd[:, :])
            nc.sync.dma_start(out=dec_sb, in_=decd[:, :])
            nc.sync.dma_start(out=enc_sb, in_=encd[:, :])
            nc.vector.memset(ones_f, 1.0)
            nc.vector.memset(eps_sb, EPS)
            nc.vector.tensor_copy(out=ones64, in_=ones_f[0:1, :])

            # pad masks from token ids
            for tok_sb, bias_sb, want_mscale in (
                (dec_sb, bias_dec, True),
                (enc_sb, bias_enc, False),
            ):
                tokf = p4.tile([P, NT], F32, tag="tokf")
                nc.vector.tensor_copy(out=tokf, in_=tok_sb)
                is0 = p4.tile([P, NT], F32, tag="is0")
                nc.vector.tensor_scalar(out=is0, in0=tokf, scalar1=0.0,
                                        scalar2=None, op0=OP.is_equal)
                nc.vector.tensor_scalar(out=bias_sb, in0=is0, scalar1=NEG,
                                        scalar2=None, op0=OP.mult)
                if want_mscale:
                    nc.scalar.activation(out=mscale, in_=is0, func=AF.Copy,
                                         bias=1.0, scale=-1.0)

            # ones column of vext (written once; V eviction never touches it)
            for i in range(NT):
                nc.vector.tensor_copy(
                    out=vext[:, i, :, DK:DK + 1],
                    in_=ones_f[:, 0:H].rearrange("p (h o) -> p h o", o=1))

            # ---------------- embedding + pe ----------------
            for tt in range(NT):
                g = p3.tile([P, D], F32, tag="tmp")
                nc.gpsimd.indirect_dma_start(
                    out=g, out_offset=None, in_=embd[:, :],
                    in_offset=IndirectOffsetOnAxis(ap=dec_sb[:, tt:tt + 1], axis=0))
                pe_t = p3.tile([P, D], F32, tag="tmp")
                nc.sync.dma_start(out=pe_t, in_=ped[tt * P:(tt + 1) * P, :])
                g2 = p3.tile([P, D], F32, tag="tmp")
                nc.vector.tensor_scalar(out=g2, in0=g, scalar1=mscale[:, tt:tt + 1],
                                        scalar2=None, op0=OP.mult)
                nc.vector.tensor_add(out=x_res[:, tt, :], in0=g2, in1=pe_t)

            def transpose_to(dst, src_tile, tt_range=range(NT)):
                # dst: [P, ND, T] f32r; src_tile(tt) -> [P, D] f32 AP
                for tt in tt_range:
                    for d in range(ND):
                        ps_t = psB.tile([P, P], F32, tag="b")
                        nc.tensor.transpose(out=ps_t,
                                            in_=src_tile(tt)[:, d * P:(d + 1) * P],
                                            identity=ident_sb)
                        nc.vector.tensor_copy(
                            out=dst[:, d, tt * P:(tt + 1) * P], in_=ps_t)

            transpose_to(xT, lambda tt: x_res[:, tt, :])

            for tt in range(NT):
                e_t = p3.tile([P, D], F32, tag="tmp")
                nc.sync.dma_start(out=e_t, in_=encxd[tt * P:(tt + 1) * P, :])
                for d in range(ND):
                    ps_t = psB.tile([P, P], F32, tag="b")
                    nc.tensor.transpose(out=ps_t, in_=e_t[:, d * P:(d + 1) * P],
                                        identity=ident_sb)
                    nc.vector.tensor_copy(out=encT[:, d, tt * P:(tt + 1) * P],
                                          in_=ps_t)

            # ---------------- helpers ----------------
            def ln_into_xres(ps_in, tt):
                pre = p3.tile([P, D], F32, tag="tmp")
                nc.vector.tensor_add(out=pre, in0=ps_in, in1=x_res[:, tt, :])
                st = p4.tile([P, nc.vector.BN_STATS_DIM], F32, tag="st")
                nc.vector.bn_stats(out=st, in_=pre)
                mv = p4.tile([P, nc.vector.BN_AGGR_DIM], F32, tag="mv")
                nc.vector.bn_aggr(out=mv, in_=st)
                std = p4.tile([P, 1], F32, tag="sd")
                nc.scalar.activation(out=std, in_=mv[:, 1:2], func=AF.Sqrt,
                                     bias=eps_sb, scale=1.0)
                rstd = p4.tile([P, 1], F32, tag="rs")
                nc.vector.reciprocal(out=rstd, in_=std)
                nc.vector.tensor_scalar(out=x_res[:, tt, :], in0=pre,
                                        scalar1=mv[:, 0:1], scalar2=rstd,
                                        op0=OP.subtract, op1=OP.mult)

            def load_wattn(wd, l):
                w = dbl.tile([P, ND, D], F32R, tag="wattn")
                nc.sync.dma_start(
                    out=w, in_=wd[l].rearrange("(kc kp) n -> kp kc n", kp=P))
                return w

            def attn(l, is_self):
                wq = load_wattn(WQS if is_self else WQC, l)
                wk = load_wattn(WKS if is_self else WKC, l)
                wv = load_wattn(WVS if is_self else WVC, l)
                wo = load_wattn(WOS if is_self else WOC, l)
                kv = xT if is_self else encT
                bias_sb = bias_dec if is_self else bias_enc

                # QT / KT projections: psum [dq_tile, t_chunk]
                for dst, w, src in ((QT, wq, xT), (KT, wk, kv)):
                    for dq in range(ND):
                        for c in range(2):
                            ps = psS.tile([P, 512], F32, tag="s")
                            for kc in range(ND):
                                nc.tensor.matmul(
                                    ps, w[:, kc, dq * P:(dq + 1) * P],
                                    src[:, kc, c * 512:(c + 1) * 512],
                                    start=(kc == 0), stop=(kc == ND - 1))
                            nc.vector.tensor_copy(
                                out=dst[:, dq, c * 512:(c + 1) * 512], in_=ps)

                # V projection: psum [k_tile(tokens), dv 512] -> vext
                for i in range(NT):
                    ps = psS.tile([P, 512], F32, tag="s")
                    for kc in range(ND):
                        nc.tensor.matmul(ps, kv[:, kc, i * P:(i + 1) * P],
                                         wv[:, kc, :],
                                         start=(kc == 0), stop=(kc == ND - 1))
                    nc.vector.tensor_copy(
                        out=vext[:, i, :, 0:DK],
                        in_=ps.rearrange("p (h v) -> p h v", h=H))

                # scores -> exp -> AV, per (q_chunk, head pair)
                for c in range(2):
                    ctx_pairs = [p5.tile([P, 512], F32R, tag="ctx",
                                          name=f"ctxp_{l}_{is_self}_{c}_{d}")
                                 for d in range(ND)]
                    for d in range(ND):
                        for hh in range(2):
                            h = 2 * d + hh
                            hsl = slice(hh * 64, (hh + 1) * 64)
                            klist = list(range(4 * (c + 1))) if is_self \
                                else list(range(NT))
                            ps_ctx = psB.tile([DK + 1, 512], F32, tag="b")
                            for ki, i in enumerate(klist):
                                ps_s = psS.tile([P, 512], F32, tag="s")
                                nc.tensor.matmul(
                                    ps_s, KT[hsl, d, i * P:(i + 1) * P],
                                    QT[hsl, d, c * 512:(c + 1) * 512],
                                    start=True, stop=True)
                                if is_self and i >= 4 * c:
                                    r = i - 4 * c
                                    nc.vector.tensor_add(
                                        out=ps_s, in0=ps_s,
                                        in1=causal_sb[:, 384 - 128 * r:
                                                      384 - 128 * r + 512])
                                e = p3.tile([P, 512], F32R, tag="exp")
                                nc.scalar.activation(
                                    out=e, in_=ps_s, func=AF.Exp,
                                    bias=bias_sb[:, i:i + 1], scale=0.125)
                                nc.tensor.matmul(
                                    ps_ctx, vext[:, i, h, :], e,
                                    start=(ki == 0), stop=(ki == len(klist) - 1))
                            recip = p3.tile([1, 512], F32R, tag="recip")
                            nc.vector.reciprocal(out=recip,
                                                 in_=ps_ctx[DK:DK + 1, :])
                            ps_r = psR.tile([64, 512], F32, tag="r")
                            nc.tensor.matmul(ps_r, ones64, recip,
                                             start=True, stop=True)
                            ctxe = p3.tile([64, 512], F32, tag="ctxe")
                            nc.vector.tensor_copy(out=ctxe, in_=ps_ctx[0:DK, :])
                            nc.vector.tensor_mul(out=ctx_pairs[d][hsl, :],
                                                 in0=ctxe, in1=ps_r)
                    # output projection + residual + LN for this chunk
                    for ts_ in range(4):
                        tt = 4 * c + ts_
                        ps_o = psB.tile([P, 512], F32, tag="b")
                        for d in range(ND):
                            nc.tensor.matmul(
                                ps_o, ctx_pairs[d][:, ts_ * P:(ts_ + 1) * P],
                                wo[:, d, :], start=(d == 0), stop=(d == ND - 1))
                        ln_into_xres(ps_o, tt)
                transpose_to(xT, lambda tt: x_res[:, tt, :])

            def ffn(l, last):
                for c in range(2):
                    ps_fs = [psB.tile([P, 512], F32, tag="b",
                                      name=f"psf_{l}_{c}_{i}") for i in range(4)]
                    for j in range(NJ):
                        if j % 2 == 0:
                            jc = j // 2
                            w1t = dbl.tile([P, ND, 256], F32R, tag="w1c")
                            nc.sync.dma_start(
                                out=w1t,
                                in_=W1D[l][:, jc * 256:(jc + 1) * 256]
                                .rearrange("(kc kp) n -> kp kc n", kp=P))
                            w2t = dbl.tile([P, 2, D], F32R, tag="w2c")
                            nc.sync.dma_start(
                                out=w2t,
                                in_=W2D[l][jc * 256:(jc + 1) * 256, :]
                                .rearrange("(jj kp) n -> kp jj n", kp=P))
                        ps_h = psS.tile([P, 512], F32, tag="s")
                        for kc in range(ND):
                            nc.tensor.matmul(
                                ps_h, w1t[:, kc, (j % 2) * P:(j % 2 + 1) * P],
                                xT[:, kc, c * 512:(c + 1) * 512],
                                start=(kc == 0), stop=(kc == ND - 1))
                        hT = p3.tile([P, 512], F32R, tag="hT")
                        nc.scalar.activation(out=hT, in_=ps_h, func=AF.Relu)
                        for ts_ in range(4):
                            nc.tensor.matmul(
                                ps_fs[ts_], hT[:, ts_ * P:(ts_ + 1) * P],
                                w2t[:, j % 2, :],
                                start=(j == 0), stop=(j == NJ - 1))
                    for ts_ in range(4):
                        ln_into_xres(ps_fs[ts_], 4 * c + ts_)
                    if not last:
                        transpose_to(xT, lambda tt: x_res[:, tt, :],
                                     tt_range=range(4 * c, 4 * c + 4))

            # ---------------- layers ----------------
            for rep in range(repeat):
                for l in range(n_layers):
                    attn(l, True)
                    attn(l, False)
                    ffn(l, last=(rep == repeat - 1 and l == n_layers - 1))

            for tt in range(NT):
                nc.sync.dma_start(out=outd[tt * P:(tt + 1) * P, :],
                                  in_=x_res[:, tt, :])

    nc.compile()
    return nc


_CACHE = {}


def get_nc(n_layers=L_FULL, repeat=1):
    key = (n_layers, repeat)
    if key not in _CACHE:
        _CACHE[key] = build(n_layers, repeat)
    return _CACHE[key]


def make_in_maps(dec_inputs, enc_inputs, enc_outputs, emb,
                 Wq_self, Wk_self, Wv_self, Wo_self,
                 Wq_cross, Wk_cross, Wv_cross, Wo_cross, W1, W2,
                 n_layers=L_FULL):
    f = np.ascontiguousarray
    emb = f(np.asarray(emb, dtype=np.float32))
    dec = np.asarray(dec_inputs).astype(np.int32)
    enc = np.asarray(enc_inputs).astype(np.int32)
    encx = f(np.asarray(enc_outputs, dtype=np.float32))
    pe = _pe_table()
    caus = _causal_master()
    ident = np.eye(P, dtype=np.float32)
    ws = {}
    for name, w in (("wq_s", Wq_self), ("wk_s", Wk_self), ("wv_s", Wv_self),
                    ("wo_s", Wo_self), ("wq_c", Wq_cross), ("wk_c", Wk_cross),
                    ("wv_c", Wv_cross), ("wo_c", Wo_cross), ("w1", W1),
                    ("w2", W2)):
        ws[name] = f(np.asarray(w, dtype=np.float32)[:n_layers])
    B = dec.shape[0]
    in_maps = []
    for b in range(B):
        m = dict(emb=emb, pe=pe, causal=caus, ident=ident,
                 dec_idx=f(dec[b].reshape(NT, P).T),
                 enc_idx=f(enc[b].reshape(NT, P).T),
                 encx=encx[b], **ws)
        in_maps.append(m)
    return in_maps


def kernel(**inputs):
    n_layers = inputs.pop("_n_layers", L_FULL)
    nc = get_nc(n_layers)
    in_maps = make_in_maps(**inputs, n_layers=n_layers)
    res = run_bass_kernel_spmd(nc, in_maps, core_ids=list(range(len(in_maps))))
    out = np.stack([r["out"] for r in res.results], axis=0)
    return out
